# revision 7
# baseline (speedup 1.0000x reference)
"""Trainium2 Bass kernel for nn_LinearAttention (RoPE(Q) @ RoPE(Q)^T @ V).

Key algebraic insight: there is no softmax, so
    out = (QR @ QR^T) @ V  ==  QR @ (QR^T @ V)
which replaces the [T,T] score matrix with a [d,d] (64x64) intermediate:
~32x fewer FLOPs. Sharding: 16 heads / 8 cores = 2 heads per core,
no cross-core communication.

All inputs are packed host-side into ONE DRAM tensor laid out exactly as
SBUF wants it ([128 partitions, chunks, 64]), so a single fully
contiguous DMA loads everything. The t-axis is permuted into chunks
(t = p*16 + c); this is valid because the contraction sums over all t
and the second matmul is row-local in t, as long as every tensor
(cos/sin tables and the output) uses the same permutation.

Per head, per core:
  1. RoPE on Q (5 elementwise DVE ops across all 16 chunks at once).
  2. S = sum_c QR_c^T @ V_c            (16 accumulating matmuls -> PSUM [64,64])
  3. QRT_c = PE-transpose(QR_c)        (16 transposes -> [64,128] tiles)
  4. out_c = QRT_c^T @ S = QR_c @ S    (16 matmuls -> PSUM [128,64])
  5. Copy to the packed out buffer; one DMA stores both heads.

This compiler build allows only ONE sync-wait per engine instruction and
Tile's wait elision is per-engine (not transitive), so the kernel is
structured to keep every instruction at <=1 wait: a single input DMA
(one DMAHW lane), a dummy PE transpose that makes the PE engine observe
the DMA semaphore once, and DVE-produced tiles for everything else.
"""

from contextlib import ExitStack

import numpy as np

import concourse.bass as bass
import concourse.mybir as mybir
import concourse.tile as tile
from concourse.bass_utils import run_bass_kernel_spmd

H, T, D = 16, 2048, 64
N_CORES = 8
HPC = H // N_CORES  # heads per core
P = 128
NT = T // P  # 16 t-chunks per head
# packed chunk layout per partition: Q0 V0 Q1 V1 COS SIN IDT(2 chunks)
NCHUNK = 4 * NT + 2 * NT + 2
F32 = mybir.dt.float32


def _rope_tables():
    inv_freq = 1.0 / (10000.0 ** (np.arange(0, D, 2, dtype=np.float32) / D))
    t = np.arange(T, dtype=np.float32)
    freqs = np.outer(t, inv_freq).astype(np.float32)  # [T, D/2]
    emb = np.concatenate([freqs, freqs], axis=-1)  # [T, D]
    return np.cos(emb).astype(np.float32), np.sin(emb).astype(np.float32)


def _build_nc():
    nc = bass.Bass()
    PACK = nc.declare_dram_parameter("PACK", [P, NCHUNK * D], F32, isOutput=False)
    OUT = nc.declare_dram_parameter("OUT", [P, HPC * NT * D], F32, isOutput=True)

    with tile.TileContext(nc) as tc, ExitStack() as ctx:
        singles = ctx.enter_context(tc.tile_pool(name="singles", bufs=1))
        hbuf = ctx.enter_context(tc.tile_pool(name="hbuf", bufs=2))
        psum = ctx.enter_context(tc.tile_pool(name="psum", bufs=2, space="PSUM"))
        psum3 = ctx.enter_context(tc.tile_pool(name="psum3", bufs=3, space="PSUM"))

        pack_sb = singles.tile([P, NCHUNK, D], F32)
        nc.sync.dma_start(out=pack_sb, in_=PACK[:].rearrange("p (c d) -> p c d", d=D))

        def chunks(c0, n):
            return pack_sb[:, c0 : c0 + n, :]

        cos_sb = chunks(4 * NT, NT)
        sin_sb = chunks(5 * NT, NT)
        idt = pack_sb[:, 6 * NT : 6 * NT + 2, :].rearrange("p c d -> p (c d)")

        # Dummy transpose: the first PE instruction, depending ONLY on the
        # input DMA, so the PE engine observes the DMA semaphore here once;
        # all later PE reads of pack_sb need no further DMA wait.
        warm_ps = psum3.tile([D, P], F32, tag="tp")
        nc.tensor.transpose(warm_ps, pack_sb[:, 0, :], idt)

        out_sb = singles.tile([P, HPC, NT, D], F32)

        for h in range(HPC):
            q_sb = chunks(h * 2 * NT, NT)
            v_sb = chunks(h * 2 * NT + NT, NT)

            # --- RoPE: qr = q * cos + rotate_half(q) * sin ---
            qr_sb = hbuf.tile([P, NT, D], F32, tag="qr")
            tmp = hbuf.tile([P, NT, D // 2], F32, tag="tmp")
            nc.vector.tensor_mul(qr_sb, q_sb, cos_sb)
            nc.vector.tensor_mul(tmp, q_sb[:, :, D // 2 :], sin_sb[:, :, : D // 2])
            nc.vector.tensor_sub(qr_sb[:, :, : D // 2], qr_sb[:, :, : D // 2], tmp)
            tmp2 = hbuf.tile([P, NT, D // 2], F32, tag="tmp2")
            nc.vector.tensor_mul(tmp2, q_sb[:, :, : D // 2], sin_sb[:, :, D // 2 :])
            nc.vector.tensor_add(qr_sb[:, :, D // 2 :], qr_sb[:, :, D // 2 :], tmp2)

            # --- Phase 2: S = QR^T @ V  (accumulate over the 16 t-chunks) ---
            s_ps = psum.tile([D, D], F32, tag="s")
            for c in range(NT):
                nc.tensor.matmul(
                    s_ps,
                    lhsT=qr_sb[:, c, :],
                    rhs=v_sb[:, c, :],
                    start=(c == 0),
                    stop=(c == NT - 1),
                )
            s_sb = hbuf.tile([D, D], F32, tag="ssb")
            nc.vector.tensor_copy(out=s_sb, in_=s_ps)

            # --- Phase 3: out_c = QR_c @ S via PE transpose of each chunk ---
            for c in range(NT):
                tp_ps = psum3.tile([D, P], F32, tag="tp")
                nc.tensor.transpose(tp_ps, qr_sb[:, c, :], idt)
                qrt_sb = hbuf.tile([D, P], F32, tag="qrt")
                nc.vector.tensor_copy(out=qrt_sb, in_=tp_ps)
                o_ps = psum3.tile([P, D], F32, tag="o")
                nc.tensor.matmul(o_ps, lhsT=qrt_sb, rhs=s_sb, start=True, stop=True)
                nc.vector.tensor_copy(out=out_sb[:, h, c, :], in_=o_ps)

        nc.sync.dma_start(
            out=OUT[:].rearrange("p (h c d) -> p h c d", h=HPC, d=D), in_=out_sb
        )

    _split_multi_waits(nc)
    return nc


def _split_multi_waits(nc):
    """This compiler build rejects instructions carrying more than one
    sync-wait command. Tile's kernel-tail drain aggregates one wait per
    live semaphore, so split the extras into single-wait NoOps placed
    immediately before it on the same engine (sequential execution on the
    engine's queue preserves the barrier semantics)."""
    n = 0
    for f in nc.m.functions:
        for blk in f.blocks:
            new_insts = []
            for inst in blk.instructions:
                si = inst.sync_info
                waits = list(si.on_wait) if si else []
                if len(waits) > 1:
                    for w in waits[:-1]:
                        nop = mybir.InstNoOp(name=f"W-split-{n}", ins=[], outs=[])
                        n += 1
                        nop.engine = inst.engine
                        nop.sync_info = mybir.SyncInfo(on_wait=[w], on_update=[])
                        new_insts.append(nop)
                    inst.sync_info = mybir.SyncInfo(
                        on_wait=[waits[-1]], on_update=list(si.on_update)
                    )
                new_insts.append(inst)
            blk.instructions = new_insts


_NC_CACHE = None


def _get_nc():
    global _NC_CACHE
    if _NC_CACHE is None:
        _NC_CACHE = _build_nc()
    return _NC_CACHE


def _pack_inputs(Qs, Vs, cos, sin, idt):
    # [T, D] -> [P, NT, D] with t = p*NT + c
    def r(x):
        return x.reshape(P, NT, D)

    packs = []
    for core in range(N_CORES):
        h0 = core * HPC
        slabs = []
        for h in range(HPC):
            slabs.append(r(Qs[h0 + h]))
            slabs.append(r(Vs[h0 + h]))
        slabs.append(r(cos))
        slabs.append(r(sin))
        slabs.append(idt.reshape(P, 2, D))
        pack = np.concatenate(slabs, axis=1).reshape(P, NCHUNK * D)
        packs.append(np.ascontiguousarray(pack))
    return packs


def run_inner(Q, K, V, trace=False):
    del K  # the module sets KR = QR; K is unused
    Qs = np.asarray(Q, dtype=np.float32)[0]  # [H, T, D]
    Vs = np.asarray(V, dtype=np.float32)[0]
    cos, sin = _rope_tables()
    idt = np.eye(P, dtype=np.float32)
    nc = _get_nc()
    in_maps = [{"PACK": p} for p in _pack_inputs(Qs, Vs, cos, sin, idt)]
    res = run_bass_kernel_spmd(nc, in_maps, list(range(N_CORES)), trace=trace)
    outs = []
    for i in range(N_CORES):
        o = np.asarray(res.results[i]["OUT"]).reshape(P, HPC, NT, D)
        outs.append(o.transpose(1, 0, 2, 3).reshape(HPC, T, D))
    out = np.concatenate(outs, axis=0)[None]  # [1, H, T, D]
    return out.astype(np.float32), res


def kernel(Q, K, V):
    out, _ = run_inner(Q, K, V, trace=False)
    return out


# revision 9
# speedup vs baseline: 1.5465x; 1.5465x over previous
"""Trainium2 Bass kernel for nn_LinearAttention (RoPE(Q) @ RoPE(Q)^T @ V).

Key algebraic insight: there is no softmax, so
    out = (QR @ QR^T) @ V  ==  QR @ (QR^T @ V)
which replaces the [T,T] score matrix with a [d,d] (64x64) intermediate:
~32x fewer FLOPs. Sharding: 16 heads / 8 cores = 2 heads per core, no
cross-core communication.

Layout: the t-axis is permuted into 16 chunks (t = p*16 + c, p = SBUF
partition). Valid because the contraction sums over all t and the second
matmul is row-local in t; the host packs/unpacks with the same
permutation. The two heads ride in the two 64-partition "lanes" of the
128x128 PE array (head h occupies d-rows/columns 64h:64h+64), so every
matmul serves both heads at once:

  1. RoPE on Q (DVE + GpSimd share the elementwise work).
  2. S2 = sum_c [qr_c(h0)|qr_c(h1)]^T @ [v_c(h0)|v_c(h1)]
     (16 accumulating matmuls N=128; diagonal 64x64 blocks are S_h).
  3. QRT_c = PE-transpose of [qr_c(h0)|qr_c(h1)]  -> both heads' lanes.
  4. outT blocks = S_h^T @ QRT lane  (4 matmuls N=512 per head,
     row+col tile_position puts the two heads in disjoint array
     quadrants so they run concurrently).
  5. One contiguous DMA stores outT; the host undoes the transpose.

The compiler build allows only ONE sync-wait per engine instruction and
Tile's wait elision is per-engine, so: 3 input DMAs land in SBUF-native
layout (host pre-packs), tiny per-engine "absorber" ops observe each DMA
semaphore once, and cross-engine produced tiles are grouped per consumer
engine. A post-pass splits any remaining multi-wait instruction (the
kernel-tail drain) into single-wait NoOps.
"""

from contextlib import ExitStack

import numpy as np

import concourse.bass as bass
import concourse.mybir as mybir
import concourse.tile as tile
from concourse.bass_utils import run_bass_kernel_spmd

H, T, D = 16, 2048, 64
N_CORES = 8
HPC = H // N_CORES  # heads per core
P = 128
NT = T // P  # 16 t-chunks per head
HD = D // 2
NTAB = NT * HD * 2 + P  # cos32 | sin32 | idt, f32 per partition
F32 = mybir.dt.float32


def _rope_tables():
    inv_freq = 1.0 / (10000.0 ** (np.arange(0, D, 2, dtype=np.float32) / D))
    t = np.arange(T, dtype=np.float32)
    freqs = np.outer(t, inv_freq).astype(np.float32)  # [T, D/2]
    return np.cos(freqs).astype(np.float32), np.sin(freqs).astype(np.float32)


def _build_nc():
    nc = bass.Bass()
    TAB = nc.declare_dram_parameter("TAB", [P, NTAB], F32, isOutput=False)
    QVA = nc.declare_dram_parameter("QVA", [P, 8 * 2 * HPC * D], F32, isOutput=False)
    QVB = nc.declare_dram_parameter("QVB", [P, 8 * 2 * HPC * D], F32, isOutput=False)
    OUT = nc.declare_dram_parameter("OUT", [P, T], F32, isOutput=True)

    with tile.TileContext(nc) as tc, ExitStack() as ctx:
        singles = ctx.enter_context(tc.tile_pool(name="singles", bufs=1))
        ps_s = ctx.enter_context(tc.tile_pool(name="ps_s", bufs=1, space="PSUM"))
        ps_tp = ctx.enter_context(tc.tile_pool(name="ps_tp", bufs=3, space="PSUM"))
        ps_o = ctx.enter_context(tc.tile_pool(name="ps_o", bufs=2, space="PSUM"))

        tab_sb = singles.tile([P, NTAB], F32)
        # qv layout per partition: [chunk, {q,v}, head, d]
        qv_sb = singles.tile([P, NT, 2, HPC, D], F32)
        nc.sync.dma_start(out=tab_sb, in_=TAB[:])
        nc.sync.dma_start(
            out=qv_sb[:, 0:8],
            in_=QVA[:].rearrange("p (c x h d) -> p c x h d", c=8, x=2, h=HPC),
        )
        nc.sync.dma_start(
            out=qv_sb[:, 8:16],
            in_=QVB[:].rearrange("p (c x h d) -> p c x h d", c=8, x=2, h=HPC),
        )

        idt = tab_sb[:, 2 * NT * HD :]

        def cos_ap(r0, nc_, h_bcast=True):
            ap = [list(tab_sb.ap[0]), [HD, nc_], [0, HPC], [1, HD]]
            return bass.AP(tensor=tab_sb.tensor, offset=tab_sb.offset + r0 * HD, ap=ap)

        def sin_ap(r0, nc_):
            ap = [list(tab_sb.ap[0]), [HD, nc_], [0, HPC], [1, HD]]
            return bass.AP(
                tensor=tab_sb.tensor, offset=tab_sb.offset + NT * HD + r0 * HD, ap=ap
            )

        scratch = singles.tile([P, 8], F32)
        qr_sb = singles.tile([P, NT, HPC, D], F32)
        tmp1 = singles.tile([P, 8, HPC, HD], F32)
        tmp2 = singles.tile([P, 8, HPC, HD], F32)
        qrt_sb = singles.tile([P, NT * P], F32)
        s_sb = singles.tile([P, D], F32)
        outT_sb = singles.tile([P, T], F32)

        # Absorbers: one tiny op per (engine, DMA) pair so every later
        # instruction needs at most one sync wait.
        nc.vector.tensor_copy(out=scratch[:, 0:1], in_=tab_sb[:, 0:1])
        nc.gpsimd.tensor_copy(out=scratch[:, 1:2], in_=tab_sb[:, 1:2])
        warm = ps_tp.tile([P, P], F32, tag="tp")
        nc.tensor.transpose(warm, idt, idt)

        s2_ps = ps_s.tile([P, P], F32)

        for half in range(2):
            r0 = half * 8
            cs = slice(r0, r0 + 8)
            q = qv_sb[:, cs, 0]  # [P, 8, HPC, D]
            cosb, sinb = cos_ap(r0, 8), sin_ap(r0, 8)

            # RoPE: DVE does the cos half + combines; GpSimd the sin muls.
            nc.gpsimd.tensor_mul(tmp1, q[:, :, :, HD:], sinb)
            nc.gpsimd.tensor_mul(tmp2, q[:, :, :, :HD], sinb)
            qr = qr_sb[:, cs]
            nc.vector.tensor_mul(qr[:, :, :, :HD], q[:, :, :, :HD], cosb)
            nc.vector.tensor_mul(qr[:, :, :, HD:], q[:, :, :, HD:], cosb)
            nc.vector.tensor_sub(qr[:, :, :, :HD], qr[:, :, :, :HD], tmp1)
            nc.vector.tensor_add(qr[:, :, :, HD:], qr[:, :, :, HD:], tmp2)

            # PE observes this half's DMA semaphore once (result unused).
            warm2 = ps_tp.tile([P, P], F32, tag="tp")
            nc.tensor.transpose(warm2, qv_sb[:, r0, 1].rearrange("p h d -> p (h d)"), idt)

            for c in range(r0, r0 + 8):
                qr2 = qr_sb[:, c].rearrange("p h d -> p (h d)")
                v2 = qv_sb[:, c, 1].rearrange("p h d -> p (h d)")
                nc.tensor.matmul(
                    s2_ps, lhsT=qr2, rhs=v2, start=(c == 0), stop=(c == NT - 1)
                )
                tp = ps_tp.tile([P, P], F32, tag="tp")
                nc.tensor.transpose(tp, qr2, idt)
                nc.scalar.copy(out=qrt_sb[:, c * P : (c + 1) * P], in_=tp)

        # S_h lives in the diagonal 64x64 blocks; keep each in its lane.
        nc.vector.tensor_copy(out=s_sb[:D], in_=s2_ps[:D, :D])
        nc.vector.tensor_copy(out=s_sb[D:], in_=s2_ps[D:, D:])

        # PE observes the ACT semaphore once (after the last qrt copy).
        warm3 = ps_s.tile([8, P], F32, tag="w3")
        nc.tensor.transpose(warm3, qrt_sb[:, NT * P - 8 :], idt)

        # outT lane blocks: S_h^T @ QRT_h, both heads concurrent on PE.
        for i in range(4):
            o_ps = ps_o.tile([P, 512], F32, tag="o")
            for h in range(HPC):
                lane = slice(h * D, (h + 1) * D)
                blk = slice(i * 512, (i + 1) * 512)
                nc.tensor.matmul(
                    o_ps[lane],
                    lhsT=s_sb[lane],
                    rhs=qrt_sb[lane, blk],
                    start=True,
                    stop=True,
                    tile_position=(h * D, h * D),
                    skip_group_check=True,
                )
            nc.vector.tensor_copy(out=outT_sb[:, i * 512 : (i + 1) * 512], in_=o_ps)

        nc.sync.dma_start(out=OUT[:], in_=outT_sb)

    _split_multi_waits(nc)
    return nc


def _split_multi_waits(nc):
    """This compiler build rejects instructions carrying more than one
    sync-wait command. Tile's kernel-tail drain aggregates one wait per
    live semaphore, so split the extras into single-wait NoOps placed
    immediately before it on the same engine (sequential execution on the
    engine's queue preserves the barrier semantics)."""
    n = 0
    for f in nc.m.functions:
        for blk in f.blocks:
            new_insts = []
            for inst in blk.instructions:
                si = inst.sync_info
                waits = list(si.on_wait) if si else []
                if len(waits) > 1:
                    for w in waits[:-1]:
                        nop = mybir.InstNoOp(name=f"W-split-{n}", ins=[], outs=[])
                        n += 1
                        nop.engine = inst.engine
                        nop.sync_info = mybir.SyncInfo(on_wait=[w], on_update=[])
                        new_insts.append(nop)
                    inst.sync_info = mybir.SyncInfo(
                        on_wait=[waits[-1]], on_update=list(si.on_update)
                    )
                new_insts.append(inst)
            blk.instructions = new_insts


_NC_CACHE = None


def _get_nc():
    global _NC_CACHE
    if _NC_CACHE is None:
        _NC_CACHE = _build_nc()
    return _NC_CACHE


def _pack_inputs(Qs, Vs, cos32, sin32, idt):
    # [T, D] -> [P, NT, D] with t = p*NT + c
    def r(x):
        return x.reshape(P, NT, -1)

    tab = np.concatenate(
        [r(cos32).reshape(P, -1), r(sin32).reshape(P, -1), idt], axis=1
    ).astype(np.float32)
    tab = np.ascontiguousarray(tab)

    in_maps = []
    for core in range(N_CORES):
        h0 = core * HPC
        # qv[p, c, x, h, d]
        qv = np.empty((P, NT, 2, HPC, D), np.float32)
        for h in range(HPC):
            qv[:, :, 0, h] = r(Qs[h0 + h])
            qv[:, :, 1, h] = r(Vs[h0 + h])
        in_maps.append(
            {
                "TAB": tab,
                "QVA": np.ascontiguousarray(qv[:, 0:8].reshape(P, -1)),
                "QVB": np.ascontiguousarray(qv[:, 8:16].reshape(P, -1)),
            }
        )
    return in_maps


def _unpack_out(o):
    # o: [P, T] = outT; rows h*64+j, cols c-major: col = c*128 + f, t = f*16+c
    a = o.reshape(HPC, D, NT, P)  # [h, j, c, f]
    return a.transpose(0, 3, 2, 1).reshape(HPC, T, D)  # [h, t=f*16+c, j]


def run_inner(Q, K, V, trace=False):
    del K  # the module sets KR = QR; K is unused
    Qs = np.asarray(Q, dtype=np.float32)[0]  # [H, T, D]
    Vs = np.asarray(V, dtype=np.float32)[0]
    cos32, sin32 = _rope_tables()
    idt = np.eye(P, dtype=np.float32)
    nc = _get_nc()
    in_maps = _pack_inputs(Qs, Vs, cos32, sin32, idt)
    res = run_bass_kernel_spmd(nc, in_maps, list(range(N_CORES)), trace=trace)
    outs = [_unpack_out(np.asarray(res.results[i]["OUT"])) for i in range(N_CORES)]
    out = np.concatenate(outs, axis=0)[None]  # [1, H, T, D]
    return out.astype(np.float32), res


def kernel(Q, K, V):
    out, _ = run_inner(Q, K, V, trace=False)
    return out


# revision 13
# speedup vs baseline: 1.5579x; 1.0074x over previous
"""Trainium2 Bass kernel for nn_LinearAttention (RoPE(Q) @ RoPE(Q)^T @ V).

Key algebraic insight: there is no softmax, so
    out = (QR @ QR^T) @ V  ==  QR @ (QR^T @ V)
which replaces the [T,T] score matrix with a [d,d] (64x64) intermediate:
~32x fewer FLOPs. Sharding: 16 heads / 8 cores = 2 heads per core, no
cross-core communication.

Layout: the t-axis is permuted into 16 chunks (t = p*16 + c, p = SBUF
partition). Valid because the contraction sums over all t and the second
matmul is row-local in t; the host packs/unpacks with the same
permutation. The two heads ride in the two 64-partition "lanes" of the
128x128 PE array (head h occupies d-rows/columns 64h:64h+64), so every
matmul serves both heads at once:

  1. RoPE on Q (DVE + GpSimd share the elementwise work).
  2. S2 = sum_c [qr_c(h0)|qr_c(h1)]^T @ [v_c(h0)|v_c(h1)]
     (16 accumulating matmuls N=128; diagonal 64x64 blocks are S_h).
  3. QRT_c = PE-transpose of [qr_c(h0)|qr_c(h1)]  -> both heads' lanes.
  4. outT blocks = S_h^T @ QRT lane  (4 matmuls N=512 per head,
     row+col tile_position puts the two heads in disjoint array
     quadrants so they run concurrently).
  5. Four DMAs stream outT out as blocks complete; the host undoes the
     transpose during unsharding.

Perf notes baked in: matmul operands are bitcast to float32r (fp32
streams the moving operand at 2 cycles/column, fp32r at 1); a burst of
dummy transposes right after the table DMA keeps the PE busy so the HAM
clock-gate reaches 2.4 GHz before the real matmul stream; cos/sin tables
are pre-expanded over the head axis so every elementwise op has plain
strided APs (no stride-0 broadcast, which hits a DVE slow path); the
Tile kernel-tail drain+barrier is replaced with a slim per-engine-drain
+ sequencer-level barrier version (the default EVSEM butterfly costs
~8 us).

The compiler build allows only ONE sync-wait per engine instruction and
Tile's wait elision is per-engine, so: input DMAs land in SBUF-native
layout (host pre-packs), tiny per-engine "absorber" ops observe each DMA
semaphore once, and cross-engine produced tiles are grouped per consumer
engine. A post-pass splits any remaining multi-wait instruction into
single-wait NoOps.
"""

from contextlib import ExitStack

import numpy as np

import concourse.bass as bass
import concourse.mybir as mybir
import concourse.tile as tile
from concourse.bass_utils import run_bass_kernel_spmd
from concourse.vector_clock import ScopedClock

H, T, D = 16, 2048, 64
N_CORES = 8
HPC = H // N_CORES  # heads per core
P = 128
NT = T // P  # 16 t-chunks per head
HD = D // 2
# table layout per partition: cosE | sinE (each [NT, HPC, HD]) | idt
NTAB = 2 * NT * HPC * HD + P
F32 = mybir.dt.float32
F32R = mybir.dt.float32r
N_WARM = 14  # dummy transposes to spin HAM up to 2.4 GHz during the DMAs


def _rope_tables():
    inv_freq = 1.0 / (10000.0 ** (np.arange(0, D, 2, dtype=np.float32) / D))
    t = np.arange(T, dtype=np.float32)
    freqs = np.outer(t, inv_freq).astype(np.float32)  # [T, D/2]
    return np.cos(freqs).astype(np.float32), np.sin(freqs).astype(np.float32)


class _SlimTileContext(tile.TileContext):
    """TileContext whose kernel tail uses per-engine drains + a
    sequencer-level (sem-only) barrier instead of the full EVSEM
    butterfly. Semantics kept: SP's drain still waits on every live
    semaphore's final value (split into single-wait NoOps later), each
    engine's pipeline is drained before the semaphore range-clear, and a
    final sem-only barrier orders the clear before the NEFF ends."""

    def _drain_and_barrier(self, tick_clock, wait_clock):
        nc = self.nc
        drain_inst = nc.sync.drain()
        wait_clock.add_sem_waits(
            drain_inst.ins, ScopedClock({None: tick_clock.global_clock})
        )
        for eng in nc.engines.values():
            if eng.engine != mybir.EngineType.SP:
                eng.drain(fusable=False)
        nc.all_engine_barrier(sem_only=True)
        popped = nc._tile_sem_poison_stack.pop()
        assert popped is self._sem_poison
        nc.clear_and_free_semaphores(list(self.sems.allocated().values()))
        nc.all_engine_barrier(sem_only=True)


def _build_nc():
    nc = bass.Bass()
    TAB = nc.declare_dram_parameter("TAB", [P, NTAB], F32, isOutput=False)
    QA = nc.declare_dram_parameter("QA", [P, 8 * HPC * D], F32, isOutput=False)
    QB = nc.declare_dram_parameter("QB", [P, 8 * HPC * D], F32, isOutput=False)
    VA = nc.declare_dram_parameter("VA", [P, 8 * HPC * D], F32R, isOutput=False)
    VB = nc.declare_dram_parameter("VB", [P, 8 * HPC * D], F32R, isOutput=False)
    OUT = nc.declare_dram_parameter("OUT", [P, T], F32, isOutput=True)

    with _SlimTileContext(nc) as tc, ExitStack() as ctx:
        singles = ctx.enter_context(tc.tile_pool(name="singles", bufs=1))
        ps_s = ctx.enter_context(tc.tile_pool(name="ps_s", bufs=1, space="PSUM"))
        ps_tp = ctx.enter_context(tc.tile_pool(name="ps_tp", bufs=3, space="PSUM"))
        ps_o = ctx.enter_context(tc.tile_pool(name="ps_o", bufs=2, space="PSUM"))

        tab_sb = singles.tile([P, NTAB], F32)
        q_sb = singles.tile([P, NT, HPC, D], F32)
        v_sb = singles.tile([P, NT, HPC, D], F32R)
        nc.sync.dma_start(out=tab_sb, in_=TAB[:])
        nc.sync.dma_start(
            out=q_sb[:, 0:8],
            in_=QA[:].rearrange("p (c h d) -> p c h d", c=8, h=HPC),
        )
        nc.sync.dma_start(
            out=v_sb[:, 0:8],
            in_=VA[:].rearrange("p (c h d) -> p c h d", c=8, h=HPC),
        )
        nc.sync.dma_start(
            out=q_sb[:, 8:16],
            in_=QB[:].rearrange("p (c h d) -> p c h d", c=8, h=HPC),
        )
        nc.sync.dma_start(
            out=v_sb[:, 8:16],
            in_=VB[:].rearrange("p (c h d) -> p c h d", c=8, h=HPC),
        )

        nexp = NT * HPC * HD
        cosE = tab_sb[:, :nexp].rearrange("p (c h k) -> p c h k", c=NT, h=HPC)
        sinE = tab_sb[:, nexp : 2 * nexp].rearrange(
            "p (c h k) -> p c h k", c=NT, h=HPC
        )
        idt = tab_sb[:, 2 * nexp :]

        scratch = singles.tile([P, 8], F32)
        qrtmp = singles.tile([P, 8, HPC, D], F32)
        qr_r = singles.tile([P, NT, HPC, D], F32R)
        tmp1 = singles.tile([P, 8, HPC, HD], F32)
        tmp2 = singles.tile([P, 8, HPC, HD], F32)
        qrt_sb = singles.tile([P, NT * P], F32R)
        s2d = singles.tile([P, P], F32R)
        outT_sb = singles.tile([P, T], F32)

        # Absorbers: one tiny op per (engine, DMA) pair so every later
        # instruction needs at most one sync wait. The PE's absorber
        # doubles as HAM warm-up: a burst of dummy transposes keeps the
        # array busy while the q/v DMAs land, so the clock-gate opens to
        # 2.4 GHz before the first real matmul.
        idt_r = singles.tile([P, P], F32R)
        nc.vector.tensor_copy(out=idt_r, in_=idt)
        nc.gpsimd.tensor_copy(out=scratch[:, 1:2], in_=tab_sb[:, 1:2])
        for _ in range(N_WARM):
            warm = ps_tp.tile([P, P], F32R, tag="tp")
            nc.tensor.transpose(warm, idt_r, idt_r)

        s2_ps = ps_s.tile([P, P], F32)

        for half in range(2):
            r0 = half * 8
            cs = slice(r0, r0 + 8)
            q = q_sb[:, cs]  # [P, 8, HPC, D]
            cosb, sinb = cosE[:, cs], sinE[:, cs]

            # RoPE: DVE does the cos half + combines; GpSimd the sin muls.
            # The final sub/add write the float32r-typed qr tile (the
            # output cast satisfies the FP32r-producer rule for matmuls).
            nc.gpsimd.tensor_mul(tmp1, q[:, :, :, HD:], sinb)
            nc.gpsimd.tensor_mul(tmp2, q[:, :, :, :HD], sinb)
            nc.vector.tensor_mul(qrtmp[:, :, :, :HD], q[:, :, :, :HD], cosb)
            nc.vector.tensor_mul(qrtmp[:, :, :, HD:], q[:, :, :, HD:], cosb)
            qr = qr_r[:, cs]
            nc.vector.tensor_sub(qr[:, :, :, :HD], qrtmp[:, :, :, :HD], tmp1)
            nc.vector.tensor_add(qr[:, :, :, HD:], qrtmp[:, :, :, HD:], tmp2)

            # PE observes this half's v-DMA semaphore once (result unused).
            warm2 = ps_tp.tile([P, P], F32R, tag="tp")
            nc.tensor.transpose(
                warm2, v_sb[:, r0].rearrange("p h d -> p (h d)"), idt_r
            )

            for c in range(r0, r0 + 8):
                qr2 = qr_r[:, c].rearrange("p h d -> p (h d)")
                v2 = v_sb[:, c].rearrange("p h d -> p (h d)")
                nc.tensor.matmul(
                    s2_ps, lhsT=qr2, rhs=v2, start=(c == 0), stop=(c == NT - 1)
                )
                tp = ps_tp.tile([P, P], F32R, tag="tp")
                nc.tensor.transpose(tp, qr2, idt_r)
                nc.scalar.copy(out=qrt_sb[:, c * P : (c + 1) * P], in_=tp)

        # Extract the diagonal S_h blocks into a block-diagonal [128,128]
        # operand: the zero off-diagonal blocks kill the cross-head terms,
        # so one full matmul serves both heads in phase 3.
        nc.vector.tensor_scalar_mul(s2d[:D, D:], idt[:D, :D], 0.0)
        nc.vector.tensor_scalar_mul(s2d[D:, :D], idt[:D, :D], 0.0)
        nc.vector.tensor_copy(out=s2d[:D, :D], in_=s2_ps[:D, :D])
        nc.vector.tensor_copy(out=s2d[D:, D:], in_=s2_ps[D:, D:])

        # PE observes the ACT semaphore once (after the last qrt copy).
        warm3 = ps_s.tile([8, P], F32R, tag="w3")
        nc.tensor.transpose(warm3, qrt_sb[:, NT * P - 8 :], idt_r)

        # outT blocks: blockdiag(S)^T @ QRT serves both heads at once.
        for i in range(4):
            o_ps = ps_o.tile([P, 512], F32, tag="o")
            blk = slice(i * 512, (i + 1) * 512)
            nc.tensor.matmul(
                o_ps, lhsT=s2d, rhs=qrt_sb[:, blk], start=True, stop=True
            )
            nc.vector.tensor_copy(out=outT_sb[:, blk], in_=o_ps)
            nc.sync.dma_start(out=OUT[:, blk], in_=outT_sb[:, blk])

    _split_multi_waits(nc)
    return nc


def _split_multi_waits(nc):
    """This compiler build rejects instructions carrying more than one
    sync-wait command. Tile's kernel-tail drain aggregates one wait per
    live semaphore, so split the extras into single-wait NoOps placed
    immediately before it on the same engine (sequential execution on the
    engine's queue preserves the barrier semantics)."""
    n = 0
    for f in nc.m.functions:
        for blk in f.blocks:
            new_insts = []
            for inst in blk.instructions:
                si = inst.sync_info
                waits = list(si.on_wait) if si else []
                if len(waits) > 1:
                    for w in waits[:-1]:
                        nop = mybir.InstNoOp(name=f"W-split-{n}", ins=[], outs=[])
                        n += 1
                        nop.engine = inst.engine
                        nop.sync_info = mybir.SyncInfo(on_wait=[w], on_update=[])
                        new_insts.append(nop)
                    inst.sync_info = mybir.SyncInfo(
                        on_wait=[waits[-1]], on_update=list(si.on_update)
                    )
                new_insts.append(inst)
            blk.instructions = new_insts


_NC_CACHE = None


def _get_nc():
    global _NC_CACHE
    if _NC_CACHE is None:
        _NC_CACHE = _build_nc()
    return _NC_CACHE


def _pack_inputs(Qs, Vs, cos32, sin32, idt):
    # [T, X] -> [P, NT, X] with t = p*NT + c
    def r(x):
        return x.reshape(P, NT, -1)

    # expand tables over the head axis so kernel APs need no broadcast
    cosE = np.repeat(r(cos32)[:, :, None, :], HPC, axis=2)  # [P, NT, HPC, HD]
    sinE = np.repeat(r(sin32)[:, :, None, :], HPC, axis=2)
    tab = np.concatenate(
        [cosE.reshape(P, -1), sinE.reshape(P, -1), idt], axis=1
    ).astype(np.float32)
    tab = np.ascontiguousarray(tab)

    in_maps = []
    for core in range(N_CORES):
        h0 = core * HPC
        q = np.empty((P, NT, HPC, D), np.float32)
        v = np.empty((P, NT, HPC, D), np.float32)
        for h in range(HPC):
            q[:, :, h] = r(Qs[h0 + h])
            v[:, :, h] = r(Vs[h0 + h])
        in_maps.append(
            {
                "TAB": tab,
                "QA": np.ascontiguousarray(q[:, 0:8].reshape(P, -1)),
                "QB": np.ascontiguousarray(q[:, 8:16].reshape(P, -1)),
                "VA": np.ascontiguousarray(v[:, 0:8].reshape(P, -1)),
                "VB": np.ascontiguousarray(v[:, 8:16].reshape(P, -1)),
            }
        )
    return in_maps


def _unpack_out(o):
    # o: [P, T] = outT; rows h*64+j, cols c-major: col = c*128 + f, t = f*16+c
    a = o.reshape(HPC, D, NT, P)  # [h, j, c, f]
    return a.transpose(0, 3, 2, 1).reshape(HPC, T, D)  # [h, t=f*16+c, j]


def run_inner(Q, K, V, trace=False):
    del K  # the module sets KR = QR; K is unused
    Qs = np.asarray(Q, dtype=np.float32)[0]  # [H, T, D]
    Vs = np.asarray(V, dtype=np.float32)[0]
    cos32, sin32 = _rope_tables()
    idt = np.eye(P, dtype=np.float32)
    nc = _get_nc()
    in_maps = _pack_inputs(Qs, Vs, cos32, sin32, idt)
    res = run_bass_kernel_spmd(nc, in_maps, list(range(N_CORES)), trace=trace)
    outs = [_unpack_out(np.asarray(res.results[i]["OUT"])) for i in range(N_CORES)]
    out = np.concatenate(outs, axis=0)[None]  # [1, H, T, D]
    return out.astype(np.float32), res


def kernel(Q, K, V):
    out, _ = run_inner(Q, K, V, trace=False)
    return out


# revision 17
# speedup vs baseline: 1.6245x; 1.0427x over previous
"""Trainium2 Bass kernel for nn_LinearAttention (RoPE(Q) @ RoPE(Q)^T @ V).

Key algebraic insight: there is no softmax, so
    out = (QR @ QR^T) @ V  ==  QR @ (QR^T @ V)
which replaces the [T,T] score matrix with a [d,d] (64x64) intermediate:
~32x fewer FLOPs. Sharding: 16 heads / 8 cores = 2 heads per core, no
cross-core communication.

Layout: the t-axis is permuted into 16 chunks (t = p*16 + c, p = SBUF
partition). Valid because the contraction sums over all t and the second
matmul is row-local in t; the host packs/unpacks with the same
permutation. The two heads ride in the two 64-partition "lanes" of the
128x128 PE array (head h occupies d-rows/columns 64h:64h+64):

  1. RoPE on Q (DVE + GpSimd share the elementwise work; Q arrives
     pre-split into rotate-half halves so every op is 2D-contiguous).
  2. S2 = sum_c [qr_c(h0)|qr_c(h1)]^T @ [v_c(h0)|v_c(h1)]
     (16 accumulating matmuls N=128; diagonal 64x64 blocks are S_h).
  3. QRT_c = PE-transpose of [qr_c(h0)|qr_c(h1)]  -> both heads' lanes.
  4. outT blocks = blockdiag(S_h0,S_h1)^T @ QRT (4 matmuls N=512; the
     zero off-diagonal blocks kill the cross-head terms).
  5. Four DMAs stream outT out as blocks complete; the host undoes the
     transpose during unsharding.

Perf notes baked in: matmul operands are float32r end-to-end (fp32
streams the moving operand at 2 cycles/column, fp32r at 1); a burst of
dependency-free garbage transposes keeps the PE busy from the preamble
on, so the HAM clock-gate reaches 2.4 GHz before the real matmul
stream; all elementwise ops use fully contiguous 2D access patterns
(multi-dim strided APs hit a DVE slow path ~3x); the Tile kernel-tail
drain+barrier is replaced with a slim per-engine-drain + sem-only
barrier (the default EVSEM butterfly costs ~8 us).

The compiler build allows only ONE sync-wait per engine instruction and
Tile's wait elision is per-engine, so: input DMAs land in SBUF-native
layout (host pre-packs), tiny per-engine "absorber" ops observe each DMA
semaphore once, and cross-engine produced tiles are grouped per consumer
engine. A post-pass splits any remaining multi-wait instruction into
single-wait NoOps.
"""

from contextlib import ExitStack

import numpy as np

import concourse.bass as bass
import concourse.mybir as mybir
import concourse.tile as tile
from concourse.bass_utils import run_bass_kernel_spmd
from concourse.vector_clock import ScopedClock

H, T, D = 16, 2048, 64
N_CORES = 8
HPC = H // N_CORES  # heads per core
P = 128
NT = T // P  # 16 t-chunks per head
HD = D // 2
NTAB = 2 * NT * HD + P  # cos | sin ([NT, HD] each) | idt, f32 per partition
F32 = mybir.dt.float32
F32R = mybir.dt.float32r
N_WARM = 12  # dep-free garbage transposes to spin HAM up to 2.4 GHz early


def _rope_tables():
    inv_freq = 1.0 / (10000.0 ** (np.arange(0, D, 2, dtype=np.float32) / D))
    t = np.arange(T, dtype=np.float32)
    freqs = np.outer(t, inv_freq).astype(np.float32)  # [T, D/2]
    return np.cos(freqs).astype(np.float32), np.sin(freqs).astype(np.float32)


class _SlimTileContext(tile.TileContext):
    """TileContext whose kernel tail uses per-engine drains + a
    sequencer-level (sem-only) barrier instead of the full EVSEM
    butterfly. Semantics kept: SP's drain still waits on every live
    semaphore's final value (split into single-wait NoOps later), each
    engine's pipeline is drained before the semaphore range-clear, and a
    final sem-only barrier orders the clear before the NEFF ends."""

    def _drain_and_barrier(self, tick_clock, wait_clock):
        nc = self.nc
        drain_inst = nc.sync.drain()
        wait_clock.add_sem_waits(
            drain_inst.ins, ScopedClock({None: tick_clock.global_clock})
        )
        for eng in nc.engines.values():
            if eng.engine != mybir.EngineType.SP:
                eng.drain(fusable=False)
        nc.all_engine_barrier(sem_only=True)
        popped = nc._tile_sem_poison_stack.pop()
        assert popped is self._sem_poison
        nc.clear_and_free_semaphores(list(self.sems.allocated().values()))
        nc.all_engine_barrier(sem_only=True)


def _build_nc():
    nc = bass.Bass()
    TAB = nc.declare_dram_parameter("TAB", [P, NTAB], F32, isOutput=False)
    # q pre-split into rotate-half halves: [head, half, chunk, k]
    QA = nc.declare_dram_parameter("QA", [P, HPC * 2 * 8 * HD], F32, isOutput=False)
    QB = nc.declare_dram_parameter("QB", [P, HPC * 2 * 8 * HD], F32, isOutput=False)
    VA = nc.declare_dram_parameter("VA", [P, 8 * HPC * D], F32R, isOutput=False)
    VB = nc.declare_dram_parameter("VB", [P, 8 * HPC * D], F32R, isOutput=False)
    OUT = nc.declare_dram_parameter("OUT", [P, T], F32, isOutput=True)

    with _SlimTileContext(nc) as tc, ExitStack() as ctx:
        singles = ctx.enter_context(tc.tile_pool(name="singles", bufs=1))
        ps_s = ctx.enter_context(tc.tile_pool(name="ps_s", bufs=1, space="PSUM"))
        ps_tp = ctx.enter_context(tc.tile_pool(name="ps_tp", bufs=3, space="PSUM"))
        ps_o = ctx.enter_context(tc.tile_pool(name="ps_o", bufs=2, space="PSUM"))

        # Garbage-input PE warm-up: no data dependencies at all, so these
        # start right after the engine preamble and keep the PE busy
        # while the input DMAs land (HAM reaches 8/8 before real work).
        spam_src = singles.tile([P, P], F32)
        nc.gpsimd.memset(spam_src[:, 0:2], 0.0)
        for _ in range(N_WARM):
            warm = ps_tp.tile([P, P], F32, tag="tp")
            nc.tensor.transpose(warm, spam_src, spam_src)

        tab_sb = singles.tile([P, NTAB], F32)
        q_sb = singles.tile([P, HPC, 2, NT, HD], F32)
        v_sb = singles.tile([P, NT, HPC, D], F32R)
        nc.sync.dma_start(out=tab_sb, in_=TAB[:])
        nc.sync.dma_start(
            out=q_sb[:, :, :, 0:8, :],
            in_=QA[:].rearrange("p (h x c k) -> p h x c k", h=HPC, x=2, c=8),
        )
        nc.sync.dma_start(
            out=v_sb[:, 0:8],
            in_=VA[:].rearrange("p (c h d) -> p c h d", c=8, h=HPC),
        )
        nc.sync.dma_start(
            out=q_sb[:, :, :, 8:16, :],
            in_=QB[:].rearrange("p (h x c k) -> p h x c k", h=HPC, x=2, c=8),
        )
        nc.sync.dma_start(
            out=v_sb[:, 8:16],
            in_=VB[:].rearrange("p (c h d) -> p c h d", c=8, h=HPC),
        )

        idt = tab_sb[:, 2 * NT * HD :]

        qr_r = singles.tile([P, NT, HPC, 2, HD], F32R)
        qrtmp = singles.tile([P, HPC, 2, 8 * HD], F32)
        tmp1 = singles.tile([P, HPC, 8 * HD], F32)
        tmp2 = singles.tile([P, HPC, 8 * HD], F32)
        qrt_sb = singles.tile([P, NT * P], F32R)
        s2d = singles.tile([P, P], F32R)
        outT_sb = singles.tile([P, T], F32)
        scratch = singles.tile([P, 8], F32)

        # Absorbers + early table work (DVE and GpSimd observe the TAB
        # semaphore; the off-diagonal zeros of the phase-3 operand only
        # need the identity slab, so they run while waiting for Q/V).
        idt_r = singles.tile([P, P], F32R)
        nc.vector.tensor_copy(out=idt_r, in_=idt)
        nc.vector.tensor_scalar_mul(s2d[:D, D:], idt[:D, :D], 0.0)
        nc.vector.tensor_scalar_mul(s2d[D:, :D], idt[:D, :D], 0.0)
        nc.gpsimd.tensor_copy(out=scratch[:, 1:2], in_=tab_sb[:, 1:2])

        s2_ps = ps_s.tile([P, P], F32)

        for half in range(2):
            r0 = half * 8
            cs = slice(r0, r0 + 8)
            fs = slice(r0 * HD, (r0 + 8) * HD)
            cosr = tab_sb[:, fs]
            sinr = tab_sb[:, NT * HD : 2 * NT * HD][:, fs]

            # RoPE, all 2D-contiguous [128, 256] slices:
            #   qr_lo = q_lo*cos - q_hi*sin ; qr_hi = q_hi*cos + q_lo*sin
            for h in range(HPC):
                qlo = q_sb[:, h, 0, cs, :].rearrange("p c k -> p (c k)")
                qhi = q_sb[:, h, 1, cs, :].rearrange("p c k -> p (c k)")
                nc.gpsimd.tensor_mul(tmp1[:, h], qhi, sinr)
                nc.gpsimd.tensor_mul(tmp2[:, h], qlo, sinr)
                nc.vector.tensor_mul(qrtmp[:, h, 0], qlo, cosr)
                nc.vector.tensor_mul(qrtmp[:, h, 1], qhi, cosr)
                qr_lo = qr_r[:, cs, h, 0, :]
                qr_hi = qr_r[:, cs, h, 1, :]
                m_lo = qrtmp[:, h, 0].rearrange("p (c k) -> p c k", c=8)
                m_hi = qrtmp[:, h, 1].rearrange("p (c k) -> p c k", c=8)
                t1 = tmp1[:, h].rearrange("p (c k) -> p c k", c=8)
                t2 = tmp2[:, h].rearrange("p (c k) -> p c k", c=8)
                nc.vector.tensor_sub(qr_lo, m_lo, t1)
                nc.vector.tensor_add(qr_hi, m_hi, t2)

            # PE observes this half's v-DMA semaphore once (result unused).
            warm2 = ps_tp.tile([P, P], F32R, tag="tp")
            nc.tensor.transpose(
                warm2, v_sb[:, r0].rearrange("p h d -> p (h d)"), idt_r
            )

            for c in range(r0, r0 + 8):
                # lhsT free order (h, half, k) = (h, d): the head lanes.
                qr2 = qr_r[:, c].rearrange("p h x k -> p (h x k)")
                v2 = v_sb[:, c].rearrange("p h d -> p (h d)")
                nc.tensor.matmul(
                    s2_ps, lhsT=qr2, rhs=v2, start=(c == 0), stop=(c == NT - 1)
                )
                # Transpose as a REGULAR matmul with the identity as the
                # moving operand (qr_c^T @ I): the moving-operand slot
                # requires a single free dimension, which qr2 (multi-dim
                # lhsT AP) cannot satisfy in transpose mode.
                tp = ps_tp.tile([P, P], F32, tag="tp")
                nc.tensor.matmul(tp, lhsT=qr2, rhs=idt_r, start=True, stop=True)
                nc.scalar.copy(out=qrt_sb[:, c * P : (c + 1) * P], in_=tp)

        # Diagonal S_h blocks -> block-diagonal phase-3 operand.
        nc.vector.tensor_copy(out=s2d[:D, :D], in_=s2_ps[:D, :D])
        nc.vector.tensor_copy(out=s2d[D:, D:], in_=s2_ps[D:, D:])

        # PE observes the ACT semaphore once (after the last qrt copy).
        warm3 = ps_s.tile([8, P], F32R, tag="w3")
        nc.tensor.transpose(warm3, qrt_sb[:, NT * P - 8 :], idt_r)

        # outT blocks: blockdiag(S)^T @ QRT serves both heads at once.
        for i in range(4):
            o_ps = ps_o.tile([P, 512], F32, tag="o")
            blk = slice(i * 512, (i + 1) * 512)
            nc.tensor.matmul(
                o_ps, lhsT=s2d, rhs=qrt_sb[:, blk], start=True, stop=True
            )
            nc.vector.tensor_copy(out=outT_sb[:, blk], in_=o_ps)
            nc.sync.dma_start(out=OUT[:, blk], in_=outT_sb[:, blk])

    _split_multi_waits(nc)
    return nc


def _split_multi_waits(nc):
    """This compiler build rejects instructions carrying more than one
    sync-wait command. Tile's kernel-tail drain aggregates one wait per
    live semaphore, so split the extras into single-wait NoOps placed
    immediately before it on the same engine (sequential execution on the
    engine's queue preserves the barrier semantics)."""
    n = 0
    for f in nc.m.functions:
        for blk in f.blocks:
            new_insts = []
            for inst in blk.instructions:
                si = inst.sync_info
                waits = list(si.on_wait) if si else []
                if len(waits) > 1:
                    for w in waits[:-1]:
                        nop = mybir.InstNoOp(name=f"W-split-{n}", ins=[], outs=[])
                        n += 1
                        nop.engine = inst.engine
                        nop.sync_info = mybir.SyncInfo(on_wait=[w], on_update=[])
                        new_insts.append(nop)
                    inst.sync_info = mybir.SyncInfo(
                        on_wait=[waits[-1]], on_update=list(si.on_update)
                    )
                new_insts.append(inst)
            blk.instructions = new_insts


_NC_CACHE = None


def _get_nc():
    global _NC_CACHE
    if _NC_CACHE is None:
        _NC_CACHE = _build_nc()
    return _NC_CACHE


def _pack_inputs(Qs, Vs, cos32, sin32, idt):
    # [T, X] -> [P, NT, X] with t = p*NT + c
    def r(x):
        return x.reshape(P, NT, -1)

    tab = np.concatenate(
        [r(cos32).reshape(P, -1), r(sin32).reshape(P, -1), idt], axis=1
    ).astype(np.float32)
    tab = np.ascontiguousarray(tab)

    in_maps = []
    for core in range(N_CORES):
        h0 = core * HPC
        # q[p, h, half, c, k], v[p, c, h, d]
        q = np.empty((P, HPC, 2, NT, HD), np.float32)
        v = np.empty((P, NT, HPC, D), np.float32)
        for h in range(HPC):
            qh = r(Qs[h0 + h])  # [P, NT, D]
            q[:, h, 0] = qh[:, :, :HD]
            q[:, h, 1] = qh[:, :, HD:]
            v[:, :, h] = r(Vs[h0 + h])
        in_maps.append(
            {
                "TAB": tab,
                "QA": np.ascontiguousarray(q[:, :, :, 0:8].reshape(P, -1)),
                "QB": np.ascontiguousarray(q[:, :, :, 8:16].reshape(P, -1)),
                "VA": np.ascontiguousarray(v[:, 0:8].reshape(P, -1)),
                "VB": np.ascontiguousarray(v[:, 8:16].reshape(P, -1)),
            }
        )
    return in_maps


def _unpack_out(o):
    # o: [P, T] = outT; rows h*64+j, cols c-major: col = c*128 + f, t = f*16+c
    a = o.reshape(HPC, D, NT, P)  # [h, j, c, f]
    return a.transpose(0, 3, 2, 1).reshape(HPC, T, D)  # [h, t=f*16+c, j]


def run_inner(Q, K, V, trace=False):
    del K  # the module sets KR = QR; K is unused
    Qs = np.asarray(Q, dtype=np.float32)[0]  # [H, T, D]
    Vs = np.asarray(V, dtype=np.float32)[0]
    cos32, sin32 = _rope_tables()
    idt = np.eye(P, dtype=np.float32)
    nc = _get_nc()
    in_maps = _pack_inputs(Qs, Vs, cos32, sin32, idt)
    res = run_bass_kernel_spmd(nc, in_maps, list(range(N_CORES)), trace=trace)
    outs = [_unpack_out(np.asarray(res.results[i]["OUT"])) for i in range(N_CORES)]
    out = np.concatenate(outs, axis=0)[None]  # [1, H, T, D]
    return out.astype(np.float32), res


def kernel(Q, K, V):
    out, _ = run_inner(Q, K, V, trace=False)
    return out


# revision 19
# speedup vs baseline: 2.1958x; 1.3517x over previous
"""Trainium2 Bass kernel for nn_LinearAttention (RoPE(Q) @ RoPE(Q)^T @ V).

Key algebraic insight: there is no softmax, so
    out = (QR @ QR^T) @ V  ==  QR @ (QR^T @ V)
which replaces the [T,T] score matrix with a [d,d] (64x64) intermediate:
~32x fewer FLOPs. Sharding: 16 heads / 8 cores = 2 heads per core, no
cross-core communication.

Layout: the t-axis is permuted into 16 chunks (t = p*16 + c, p = SBUF
partition). Valid because the contraction sums over all t and the second
matmul is row-local in t; the host packs/unpacks with the same
permutation. The two heads ride in the two 64-partition "lanes" of the
128x128 PE array (head h occupies d-rows/columns 64h:64h+64):

  1. RoPE on Q (DVE + GpSimd share the elementwise work; Q arrives
     pre-split into rotate-half halves so every op is 2D-contiguous).
  2. S2 = sum_c [qr_c(h0)|qr_c(h1)]^T @ [v_c(h0)|v_c(h1)]
     (16 accumulating matmuls N=128; diagonal 64x64 blocks are S_h).
  3. QRT_c = PE-transpose of [qr_c(h0)|qr_c(h1)]  -> both heads' lanes.
  4. outT blocks = blockdiag(S_h0,S_h1)^T @ QRT (4 matmuls N=512; the
     zero off-diagonal blocks kill the cross-head terms).
  5. Four DMAs stream outT out as blocks complete; the host undoes the
     transpose during unsharding.

Perf notes baked in: matmul operands are float32r end-to-end (fp32
streams the moving operand at 2 cycles/column, fp32r at 1); a burst of
dependency-free garbage transposes keeps the PE busy from the preamble
on, so the HAM clock-gate reaches 2.4 GHz before the real matmul
stream; all elementwise ops use fully contiguous 2D access patterns
(multi-dim strided APs hit a DVE slow path ~3x); the Tile kernel-tail
drain+barrier is replaced with a slim per-engine-drain + sem-only
barrier (the default EVSEM butterfly costs ~8 us).

The compiler build allows only ONE sync-wait per engine instruction and
Tile's wait elision is per-engine, so: input DMAs land in SBUF-native
layout (host pre-packs), tiny per-engine "absorber" ops observe each DMA
semaphore once, and cross-engine produced tiles are grouped per consumer
engine. A post-pass splits any remaining multi-wait instruction into
single-wait NoOps.
"""

from contextlib import ExitStack

import numpy as np

import concourse.bass as bass
import concourse.mybir as mybir
import concourse.tile as tile
from concourse.bass_utils import run_bass_kernel_spmd
from concourse.vector_clock import ScopedClock

H, T, D = 16, 2048, 64
N_CORES = 8
HPC = H // N_CORES  # heads per core
P = 128
NT = T // P  # 16 t-chunks per head
HD = D // 2
NTAB = 2 * NT * HD + P  # cos | sin ([NT, HD] each) | idt, f32 per partition
F32 = mybir.dt.float32
F32R = mybir.dt.float32r
BF16 = mybir.dt.bfloat16
N_WARM = 12  # dep-free garbage transposes to spin HAM up to 2.4 GHz early


def _rope_tables():
    inv_freq = 1.0 / (10000.0 ** (np.arange(0, D, 2, dtype=np.float32) / D))
    t = np.arange(T, dtype=np.float32)
    freqs = np.outer(t, inv_freq).astype(np.float32)  # [T, D/2]
    return np.cos(freqs).astype(np.float32), np.sin(freqs).astype(np.float32)


class _SlimTileContext(tile.TileContext):
    """TileContext whose kernel tail uses per-engine drains + a
    sequencer-level (sem-only) barrier instead of the full EVSEM
    butterfly. Semantics kept: SP's drain still waits on every live
    semaphore's final value (split into single-wait NoOps later), each
    engine's pipeline is drained before the semaphore range-clear, and a
    final sem-only barrier orders the clear before the NEFF ends."""

    def _drain_and_barrier(self, tick_clock, wait_clock):
        nc = self.nc
        drain_inst = nc.sync.drain()
        wait_clock.add_sem_waits(
            drain_inst.ins, ScopedClock({None: tick_clock.global_clock})
        )
        for eng in nc.engines.values():
            if eng.engine != mybir.EngineType.SP:
                eng.drain(fusable=False)
        nc.all_engine_barrier(sem_only=True)
        popped = nc._tile_sem_poison_stack.pop()
        assert popped is self._sem_poison
        nc.clear_and_free_semaphores(list(self.sems.allocated().values()))
        nc.all_engine_barrier(sem_only=True)


def _build_nc():
    nc = bass.Bass()
    TAB = nc.declare_dram_parameter("TAB", [P, NTAB], BF16, isOutput=False)
    # q pre-split into rotate-half halves: [head, half, chunk, k]
    QA = nc.declare_dram_parameter("QA", [P, HPC * 2 * 8 * HD], BF16, isOutput=False)
    QB = nc.declare_dram_parameter("QB", [P, HPC * 2 * 8 * HD], BF16, isOutput=False)
    VA = nc.declare_dram_parameter("VA", [P, 8 * HPC * D], BF16, isOutput=False)
    VB = nc.declare_dram_parameter("VB", [P, 8 * HPC * D], BF16, isOutput=False)
    OUT = nc.declare_dram_parameter("OUT", [P, T], F32, isOutput=True)

    with _SlimTileContext(nc) as tc, ExitStack() as ctx:
        singles = ctx.enter_context(tc.tile_pool(name="singles", bufs=1))
        ps_s = ctx.enter_context(tc.tile_pool(name="ps_s", bufs=1, space="PSUM"))
        ps_tp = ctx.enter_context(tc.tile_pool(name="ps_tp", bufs=3, space="PSUM"))
        ps_o = ctx.enter_context(tc.tile_pool(name="ps_o", bufs=2, space="PSUM"))

        # Garbage-input PE warm-up: no data dependencies at all, so these
        # start right after the engine preamble and keep the PE busy
        # while the input DMAs land (HAM reaches 8/8 before real work).
        spam_src = singles.tile([P, P], F32)
        nc.gpsimd.memset(spam_src[:, 0:2], 0.0)
        for _ in range(N_WARM):
            warm = ps_tp.tile([P, P], F32, tag="tp")
            nc.tensor.transpose(warm, spam_src, spam_src)

        tab_sb = singles.tile([P, NTAB], BF16)
        q_sb = singles.tile([P, HPC, 2, NT, HD], BF16)
        v_sb = singles.tile([P, NT, HPC, D], BF16)
        nc.sync.dma_start(out=tab_sb, in_=TAB[:])
        nc.sync.dma_start(
            out=q_sb[:, :, :, 0:8, :],
            in_=QA[:].rearrange("p (h x c k) -> p h x c k", h=HPC, x=2, c=8),
        )
        nc.sync.dma_start(
            out=v_sb[:, 0:8],
            in_=VA[:].rearrange("p (c h d) -> p c h d", c=8, h=HPC),
        )
        nc.sync.dma_start(
            out=q_sb[:, :, :, 8:16, :],
            in_=QB[:].rearrange("p (h x c k) -> p h x c k", h=HPC, x=2, c=8),
        )
        nc.sync.dma_start(
            out=v_sb[:, 8:16],
            in_=VB[:].rearrange("p (c h d) -> p c h d", c=8, h=HPC),
        )

        idt = tab_sb[:, 2 * NT * HD :]

        qr_r = singles.tile([P, NT, HPC, 2, HD], BF16)
        qrtmp = singles.tile([P, HPC, 2, 8 * HD], BF16)
        tmp1 = singles.tile([P, HPC, 8 * HD], BF16)
        tmp2 = singles.tile([P, HPC, 8 * HD], BF16)
        qrt_sb = singles.tile([P, NT * P], BF16)
        s2d = singles.tile([P, P], BF16)
        outT_sb = singles.tile([P, T], F32)
        scratch = singles.tile([P, 8], F32)

        # Absorbers + early table work (DVE and GpSimd observe the TAB
        # semaphore; the off-diagonal zeros of the phase-3 operand only
        # need the identity slab, so they run while waiting for Q/V).
        idt_r = singles.tile([P, P], BF16)
        nc.vector.tensor_copy(out=idt_r, in_=idt)
        nc.vector.tensor_scalar_mul(s2d[:D, D:], idt[:D, :D], 0.0)
        nc.vector.tensor_scalar_mul(s2d[D:, :D], idt[:D, :D], 0.0)

        s2_ps = ps_s.tile([P, P], F32)

        for half in range(2):
            r0 = half * 8
            cs = slice(r0, r0 + 8)
            fs = slice(r0 * HD, (r0 + 8) * HD)
            cosr = tab_sb[:, fs]
            sinr = tab_sb[:, NT * HD : 2 * NT * HD][:, fs]

            # RoPE, all 2D-contiguous [128, 256] slices:
            #   qr_lo = q_lo*cos - q_hi*sin ; qr_hi = q_hi*cos + q_lo*sin
            for h in range(HPC):
                qlo = q_sb[:, h, 0, cs, :].rearrange("p c k -> p (c k)")
                qhi = q_sb[:, h, 1, cs, :].rearrange("p c k -> p (c k)")
                nc.vector.tensor_mul(tmp1[:, h], qhi, sinr)
                nc.vector.tensor_mul(tmp2[:, h], qlo, sinr)
                nc.vector.tensor_mul(qrtmp[:, h, 0], qlo, cosr)
                nc.vector.tensor_mul(qrtmp[:, h, 1], qhi, cosr)
                qr_lo = qr_r[:, cs, h, 0, :]
                qr_hi = qr_r[:, cs, h, 1, :]
                m_lo = qrtmp[:, h, 0].rearrange("p (c k) -> p c k", c=8)
                m_hi = qrtmp[:, h, 1].rearrange("p (c k) -> p c k", c=8)
                t1 = tmp1[:, h].rearrange("p (c k) -> p c k", c=8)
                t2 = tmp2[:, h].rearrange("p (c k) -> p c k", c=8)
                nc.vector.tensor_sub(qr_lo, m_lo, t1)
                nc.vector.tensor_add(qr_hi, m_hi, t2)

            # PE observes this half's v-DMA semaphore once (result unused).
            warm2 = ps_tp.tile([P, P], BF16, tag="tp")
            nc.tensor.transpose(
                warm2, v_sb[:, r0].rearrange("p h d -> p (h d)"), idt_r
            )

            for c in range(r0, r0 + 8):
                # lhsT free order (h, half, k) = (h, d): the head lanes.
                qr2 = qr_r[:, c].rearrange("p h x k -> p (h x k)")
                v2 = v_sb[:, c].rearrange("p h d -> p (h d)")
                nc.tensor.matmul(
                    s2_ps, lhsT=qr2, rhs=v2, start=(c == 0), stop=(c == NT - 1)
                )
                # Transpose as a REGULAR matmul with the identity as the
                # moving operand (qr_c^T @ I): the moving-operand slot
                # requires a single free dimension, which qr2 (multi-dim
                # lhsT AP) cannot satisfy in transpose mode.
                tp = ps_tp.tile([P, P], F32, tag="tp")
                nc.tensor.matmul(tp, lhsT=qr2, rhs=idt_r, start=True, stop=True)
                nc.scalar.copy(out=qrt_sb[:, c * P : (c + 1) * P], in_=tp)

        # Diagonal S_h blocks -> block-diagonal phase-3 operand.
        nc.vector.tensor_copy(out=s2d[:D, :D], in_=s2_ps[:D, :D])
        nc.vector.tensor_copy(out=s2d[D:, D:], in_=s2_ps[D:, D:])

        # PE observes the ACT semaphore once (after the last qrt copy).
        warm3 = ps_s.tile([8, P], BF16, tag="w3")
        nc.tensor.transpose(warm3, qrt_sb[:, NT * P - 8 :], idt_r)

        # outT blocks: blockdiag(S)^T @ QRT serves both heads at once.
        for i in range(4):
            o_ps = ps_o.tile([P, 512], F32, tag="o")
            blk = slice(i * 512, (i + 1) * 512)
            nc.tensor.matmul(
                o_ps, lhsT=s2d, rhs=qrt_sb[:, blk], start=True, stop=True
            )
            nc.vector.tensor_copy(out=outT_sb[:, blk], in_=o_ps)
            nc.sync.dma_start(out=OUT[:, blk], in_=outT_sb[:, blk])

    _split_multi_waits(nc)
    return nc


def _split_multi_waits(nc):
    """This compiler build rejects instructions carrying more than one
    sync-wait command. Tile's kernel-tail drain aggregates one wait per
    live semaphore, so split the extras into single-wait NoOps placed
    immediately before it on the same engine (sequential execution on the
    engine's queue preserves the barrier semantics)."""
    n = 0
    for f in nc.m.functions:
        for blk in f.blocks:
            new_insts = []
            for inst in blk.instructions:
                si = inst.sync_info
                waits = list(si.on_wait) if si else []
                if len(waits) > 1:
                    for w in waits[:-1]:
                        nop = mybir.InstNoOp(name=f"W-split-{n}", ins=[], outs=[])
                        n += 1
                        nop.engine = inst.engine
                        nop.sync_info = mybir.SyncInfo(on_wait=[w], on_update=[])
                        new_insts.append(nop)
                    inst.sync_info = mybir.SyncInfo(
                        on_wait=[waits[-1]], on_update=list(si.on_update)
                    )
                new_insts.append(inst)
            blk.instructions = new_insts


_NC_CACHE = None


def _get_nc():
    global _NC_CACHE
    if _NC_CACHE is None:
        _NC_CACHE = _build_nc()
    return _NC_CACHE


def _pack_inputs(Qs, Vs, cos32, sin32, idt):
    import ml_dtypes

    bf16 = ml_dtypes.bfloat16

    # [T, X] -> [P, NT, X] with t = p*NT + c
    def r(x):
        return x.reshape(P, NT, -1)

    tab = np.concatenate(
        [r(cos32).reshape(P, -1), r(sin32).reshape(P, -1), idt], axis=1
    ).astype(bf16)
    tab = np.ascontiguousarray(tab)

    in_maps = []
    for core in range(N_CORES):
        h0 = core * HPC
        # q[p, h, half, c, k], v[p, c, h, d]
        q = np.empty((P, HPC, 2, NT, HD), np.float32)
        v = np.empty((P, NT, HPC, D), np.float32)
        for h in range(HPC):
            qh = r(Qs[h0 + h])  # [P, NT, D]
            q[:, h, 0] = qh[:, :, :HD]
            q[:, h, 1] = qh[:, :, HD:]
            v[:, :, h] = r(Vs[h0 + h])
        in_maps.append(
            {
                "TAB": tab,
                "QA": np.ascontiguousarray(q[:, :, :, 0:8].reshape(P, -1).astype(bf16)),
                "QB": np.ascontiguousarray(q[:, :, :, 8:16].reshape(P, -1).astype(bf16)),
                "VA": np.ascontiguousarray(v[:, 0:8].reshape(P, -1).astype(bf16)),
                "VB": np.ascontiguousarray(v[:, 8:16].reshape(P, -1).astype(bf16)),
            }
        )
    return in_maps


def _unpack_out(o):
    # o: [P, T] = outT; rows h*64+j, cols c-major: col = c*128 + f, t = f*16+c
    a = o.reshape(HPC, D, NT, P)  # [h, j, c, f]
    return a.transpose(0, 3, 2, 1).reshape(HPC, T, D)  # [h, t=f*16+c, j]


def run_inner(Q, K, V, trace=False):
    del K  # the module sets KR = QR; K is unused
    Qs = np.asarray(Q, dtype=np.float32)[0]  # [H, T, D]
    Vs = np.asarray(V, dtype=np.float32)[0]
    cos32, sin32 = _rope_tables()
    idt = np.eye(P, dtype=np.float32)
    nc = _get_nc()
    in_maps = _pack_inputs(Qs, Vs, cos32, sin32, idt)
    res = run_bass_kernel_spmd(nc, in_maps, list(range(N_CORES)), trace=trace)
    outs = [_unpack_out(np.asarray(res.results[i]["OUT"])) for i in range(N_CORES)]
    out = np.concatenate(outs, axis=0)[None]  # [1, H, T, D]
    return out.astype(np.float32), res


def kernel(Q, K, V):
    out, _ = run_inner(Q, K, V, trace=False)
    return out
